# revision 3
# baseline (speedup 1.0000x reference)
"""EquiAttention Trainium2 kernel.

Computes the reference nn_EquiAttention forward pass on 8 NeuronCores,
data-parallel over the batch axis (64 batches -> 8 per core).

Math refactoring (validated exact in float64):
  The reference builds q/k embeddings of width 192:
    q = [ (Wq @ vecs).flat (128) , scalars @ Wq_s.T + bq_s (64) ]
    k = [ (Wk @ vecs * METRIC).flat (128) , scalars @ Wk_s.T + bk_s (64) ]
  Softmax over keys is invariant to per-query constants, so scores fold
  to a 128-dim contraction plus a per-key bias:
    scores[q,m] ~ qv_q.kv_m + s_q.(H s_m) + c2.s_m    (mod per-q const)
  with  qv = vecs.flat (64),  kv[(j,k),m] = scale*METRIC[k]*(G @ vecs[..,k])
        G = Wq.T @ Wk,  H = scale * Wq_s.T @ Wk_s,  c2 = scale * Wk_s.T @ bq_s
  The per-key bias folds into V:  w_m = exp(c2.s_m) (values ~[0.6,1.6]),
  Vaug[m] = [w_m * v_m, w_m];  out = acc[:, :64] / acc[:, 64].

Per-core layout: qT/kT hold the embeddings transposed (feature dim on
partitions) so scores tiles are direct PE matmuls; P chunks are PE-
transposed for the P @ Vaug contraction.
"""

import numpy as np

B, N = 64, 1024
NCORES = 8
BL = B // NCORES          # batches per core
NB = N // 128             # 128-row blocks per sequence
SCALE = 1.0 / np.sqrt(192.0)

_CACHE = {}


def _build_program():
    import concourse.bacc as bacc
    import concourse.tile as tile
    from concourse import mybir

    f32 = mybir.dt.float32

    nc = bacc.Bacc("TRN2", target_bir_lowering=False,
                   debug=False, num_devices=NCORES)

    aps = {
        "vectors": nc.dram_tensor("vectors", [BL, N, 64], f32,
                                  kind="ExternalInput").ap(),
        "scalars": nc.dram_tensor("scalars", [BL, N, 64], f32,
                                  kind="ExternalInput").ap(),
        "BD": nc.dram_tensor("BD", [128, 128], f32, kind="ExternalInput").ap(),
        "WvC2": nc.dram_tensor("WvC2", [128, 65], f32, kind="ExternalInput").ap(),
        "out": nc.dram_tensor("out", [BL, N, 64], f32, kind="ExternalOutput").ap(),
    }

    with tile.TileContext(nc) as tc:
        _emit(tc, aps)

    nc.compile()
    return nc


def _emit(tc, aps):
    from contextlib import ExitStack
    import concourse.bass as bass
    import concourse.masks as masks
    from concourse import mybir

    nc = tc.nc
    f32 = mybir.dt.float32
    PS = "PSUM"
    Act = mybir.ActivationFunctionType
    Alu = mybir.AluOpType
    X = mybir.AxisListType.X

    vecs_d, scal_d = aps["vectors"], aps["scalars"]
    bd_d, wvc2_d, out_d = aps["BD"], aps["WvC2"], aps["out"]

    with ExitStack() as ctx:
        singles = ctx.enter_context(tc.tile_pool(name="singles", bufs=1))
        raw = ctx.enter_context(tc.tile_pool(name="raw", bufs=2))
        emb = ctx.enter_context(tc.tile_pool(name="emb", bufs=2))
        small = ctx.enter_context(tc.tile_pool(name="small", bufs=4))
        pP = ctx.enter_context(tc.tile_pool(name="pP", bufs=2))
        pPT = ctx.enter_context(tc.tile_pool(name="pPT", bufs=2))
        outp = ctx.enter_context(tc.tile_pool(name="outp", bufs=2))
        psS = ctx.enter_context(tc.tile_pool(name="psS", bufs=3, space=PS))
        psPT = ctx.enter_context(tc.tile_pool(name="psPT", bufs=2, space=PS))
        psOut = ctx.enter_context(tc.tile_pool(name="psOut", bufs=1, space=PS))
        psEmb = ctx.enter_context(tc.tile_pool(name="psEmb", bufs=2, space=PS))

        ident = singles.tile([128, 128], f32)
        masks.make_identity(nc, ident[:])
        bd = singles.tile([128, 128], f32)
        nc.sync.dma_start(out=bd[:], in_=bd_d[:, :])
        wvc2 = singles.tile([128, 65], f32)
        nc.sync.dma_start(out=wvc2[:], in_=wvc2_d[:, :])

        for b in range(BL):
            # ---------------- embedding phase ----------------
            vraw = raw.tile([128, NB, 64], f32, tag="vraw")
            nc.sync.dma_start(out=vraw[:],
                              in_=vecs_d[b].rearrange("(c p) f -> p c f", p=128))
            sraw = raw.tile([128, NB, 64], f32, tag="sraw")
            nc.sync.dma_start(out=sraw[:],
                              in_=scal_d[b].rearrange("(c p) f -> p c f", p=128))

            # Lorentz normalization of the 16 four-vectors per particle
            sq = raw.tile([128, NB, 16, 4], f32, tag="sq")
            nc.vector.tensor_mul(sq[:], vraw[:], vraw[:])
            nrm = raw.tile([128, NB, 16], f32, tag="nrm")
            nc.vector.tensor_sub(nrm[:], sq[:, :, :, 0], sq[:, :, :, 1])
            nc.vector.tensor_sub(nrm[:], nrm[:], sq[:, :, :, 2])
            nc.vector.tensor_sub(nrm[:], nrm[:], sq[:, :, :, 3])
            nc.scalar.activation(out=nrm[:], in_=nrm[:], func=Act.Abs)
            nc.vector.tensor_scalar_max(nrm[:], nrm[:], 1e-5)
            nc.scalar.activation(out=nrm[:], in_=nrm[:], func=Act.Sqrt)
            rn = raw.tile([128, NB, 16], f32, tag="rn")
            nc.vector.reciprocal(rn[:], nrm[:])
            vecs = raw.tile([128, NB, 16, 4], f32, tag="vecs")
            rn_b = bass.AP(tensor=rn.tensor, offset=rn.offset,
                           ap=[rn.ap[0], rn.ap[1], rn.ap[2], [0, 4]])
            nc.vector.tensor_mul(vecs[:],
                                 vraw[:].rearrange("p c (j k) -> p c j k", k=4),
                                 rn_b)

            # transpose into qT = [vecsT ; scalarsT]  (feature dim on partitions)
            qT = emb.tile([128, N], f32, tag="qT")
            for c in range(NB):
                cols = slice(c * 128, (c + 1) * 128)
                ptv = psEmb.tile([64, 128], f32, tag="pemb")
                nc.tensor.transpose(ptv[:],
                                    vecs[:, c].rearrange("p j k -> p (j k)"),
                                    ident[:])
                nc.any.tensor_copy(qT[0:64, cols], ptv[:])
                pts = psEmb.tile([64, 128], f32, tag="pemb")
                nc.tensor.transpose(pts[:], sraw[:, c], ident[:])
                nc.any.tensor_copy(qT[64:128, cols], pts[:])

            # kT = blockdiag(GT64, Hlhs).T @ qT  (both halves in one matmul)
            kT = emb.tile([128, N], f32, tag="kT")
            for h in range(2):
                cols = slice(h * 512, (h + 1) * 512)
                pk = psEmb.tile([128, 512], f32, tag="pemb")
                nc.tensor.matmul(pk[:], bd[:], qT[:, cols], start=True, stop=True)
                nc.any.tensor_copy(kT[:, cols], pk[:])

            # Vaug chunks: [128m, 65] = [w * (vecs @ Wv64), w]
            vaug = emb.tile([128, NB, 65], f32, tag="vaug")
            for c in range(NB):
                cols = slice(c * 128, (c + 1) * 128)
                pv = psEmb.tile([128, 65], f32, tag="pemb")
                nc.tensor.matmul(pv[:], qT[:, cols], wvc2[:], start=True, stop=True)
                wcol = small.tile([128, 1], f32, tag="wcol")
                nc.scalar.activation(out=wcol[:], in_=pv[:, 64:65], func=Act.Exp)
                nc.vector.tensor_scalar_mul(vaug[:, c, 0:64], pv[:, 0:64], wcol[:])
                nc.any.tensor_copy(vaug[:, c, 64:65], wcol[:])

            # ---------------- attention phase ----------------
            for qb in range(NB):
                qs = slice(qb * 128, (qb + 1) * 128)
                Sh = []
                for h in range(2):
                    cols = slice(h * 512, (h + 1) * 512)
                    S = psS.tile([128, 512], f32, tag="S")
                    nc.tensor.matmul(S[:], qT[:, qs], kT[:, cols],
                                     start=True, stop=True)
                    Sh.append(S)
                m0 = small.tile([128, 1], f32, tag="m0")
                m1 = small.tile([128, 1], f32, tag="m1")
                nc.vector.tensor_reduce(m0[:], Sh[0][:], axis=X, op=Alu.max,
                                        negate=True)
                nc.vector.tensor_reduce(m1[:], Sh[1][:], axis=X, op=Alu.max,
                                        negate=True)
                negmax = small.tile([128, 1], f32, tag="negmax")
                nc.vector.tensor_tensor(negmax[:], m0[:], m1[:], op=Alu.min)

                P = pP.tile([128, N], f32, tag="P")
                for h in range(2):
                    cols = slice(h * 512, (h + 1) * 512)
                    nc.scalar.activation(out=P[:, cols], in_=Sh[h][:],
                                         func=Act.Exp, bias=negmax[:], scale=1.0)

                acc = psOut.tile([128, 65], f32, tag="acc")
                for mc in range(NB):
                    ms = slice(mc * 128, (mc + 1) * 128)
                    ptp = psPT.tile([128, 128], f32, tag="ptp")
                    nc.tensor.transpose(ptp[:], P[:, ms], ident[:])
                    pts = pPT.tile([128, 128], f32, tag="pts")
                    nc.any.tensor_copy(pts[:], ptp[:])
                    nc.tensor.matmul(acc[:], pts[:], vaug[:, mc, :],
                                     start=(mc == 0), stop=(mc == NB - 1))

                rden = small.tile([128, 1], f32, tag="rden")
                nc.vector.reciprocal(rden[:], acc[:, 64:65])
                ob = outp.tile([128, 64], f32, tag="ob")
                nc.vector.tensor_scalar_mul(ob[:], acc[:, 0:64], rden[:])
                nc.sync.dma_start(out=out_d[b, qs, :], in_=ob[:])


def _host_weights(Wq, Wk, Wv, Wq_s, Wk_s, bq_s):
    """Fold the tiny EquiLinear weights (float64 precompute, cast f32)."""
    METRIC = np.array([1.0, -1.0, -1.0, -1.0], dtype=np.float64)
    G = Wq.astype(np.float64).T @ Wk.astype(np.float64)            # [16,16]
    BD = np.zeros((128, 128), dtype=np.float64)
    for k in range(4):
        # lhsT[(j',k), (j,k)] = SCALE * METRIC[k] * G[j, j']
        BD[k:64:4, k:64:4] = SCALE * METRIC[k] * G.T
    # lhsT[h, g] = SCALE * H[g, h],  H = Wq_s.T @ Wk_s
    BD[64:, 64:] = SCALE * (Wk_s.astype(np.float64).T @ Wq_s.astype(np.float64))
    E = np.exp(Wv.astype(np.float64))                              # [16,16]
    WvC2 = np.zeros((128, 65), dtype=np.float64)
    for k in range(4):
        # rhs[(j,k), (i,k)] = E[i, j]
        WvC2[k:64:4, k:64:4] = E.T
    WvC2[64:, 64] = SCALE * (Wk_s.astype(np.float64).T @ bq_s.astype(np.float64))
    return (np.ascontiguousarray(BD, dtype=np.float32),
            np.ascontiguousarray(WvC2, dtype=np.float32))


def _get_program():
    if "nc" not in _CACHE:
        _CACHE["nc"] = _build_program()
    return _CACHE["nc"]


def _prepare_in_maps(vectors, scalars, Wq, Wq_s, bq_s, Wk, Wk_s, bk_s, Wv):
    BD, WvC2 = _host_weights(Wq, Wk, Wv, Wq_s, Wk_s, bq_s)
    vecs_flat = np.ascontiguousarray(
        np.asarray(vectors).reshape(B, N, 64), dtype=np.float32)
    scal = np.ascontiguousarray(scalars, dtype=np.float32)

    in_maps = []
    for c in range(NCORES):
        sl = slice(c * BL, (c + 1) * BL)
        in_maps.append({
            "vectors": np.ascontiguousarray(vecs_flat[sl]),
            "scalars": np.ascontiguousarray(scal[sl]),
            "BD": BD,
            "WvC2": WvC2,
        })
    return in_maps


def _run(in_maps, **kw):
    from concourse.bass_utils import run_bass_kernel_spmd
    nc = _get_program()
    return run_bass_kernel_spmd(nc, in_maps, list(range(NCORES)), **kw)


def kernel(vectors, scalars, Wq, Wq_s, bq_s, Wk, Wk_s, bk_s, Wv):
    in_maps = _prepare_in_maps(vectors, scalars, Wq, Wq_s, bq_s,
                               Wk, Wk_s, bk_s, Wv)
    res = _run(in_maps)
    out = np.concatenate([res.results[c]["out"] for c in range(NCORES)], axis=0)
    return out.reshape(B, N, 16, 4).astype(np.float32)
